# revision 1
# baseline (speedup 1.0000x reference)
"""Trainium2 Bass kernel for a binarized (1w1a) BasicBlock:

    out1 = hardtanh(BN1(binconv(x, w1)))          # BN in training mode (batch stats)
    out  = hardtanh(BN2(binconv(out1, w2)) + x)   # identity shortcut

binconv(x, w) = conv3x3(sign(x), sign(w), pad=1) * (SCALE / K)

Sharding: data-parallel over batch (4 images per core on 8 cores), weights
replicated.  BN batch statistics (per-channel sum and sum-of-squares) are
combined with a tiny cross-core AllReduce.

Implementation notes:
  - sign() values (+-1, 0) are exact in fp8/bf16, and the 3x3x256 conv
    accumulates integers |S| <= 2304 in fp32 PSUM, so the convolutions are
    bit-exact at fp8 TensorE rate.  The SCALE/K factor commutes through
    BatchNorm and is folded into eps:  eps_eff = eps / (SCALE/K)^2.
  - conv3x3 = 9 shifted matmuls accumulated in PSUM, with fp8 DoubleRow
    contracting both 128-channel blocks per pass.  Activations live in SBUF
    as [128 ch-pair, 2, n, 58, 58] zero-padded images, so every shift is an
    access-pattern offset.  Each matmul reads 8 padded rows contiguously
    (8 x 58 = 464 columns); the 2 junk columns between output rows are
    simply never read back.
  - Per-channel statistics ride on ScalarE (activation accum_out fused with
    the PSUM->fp16 copy) and VectorE (fused square+reduce) while TensorE
    streams matmuls.  conv outputs are exact integers in fp16 and get
    re-binarized with a single fused sign(A*y + B) activation per tile.
"""

import numpy as np
import ml_dtypes

import concourse.bass as bass
import concourse.tile as tile
from concourse import bacc, mybir
from concourse import bass_utils

N_CORES = 8
N, C, H, W = 32, 256, 56, 56
NL = N // N_CORES          # images per core
HP = H + 2                 # padded rows (58)
WP = 64                    # padded row pitch (bytes): keeps DoubleRow offsets 16B-aligned
IMG = HP * WP              # 3712 padded image elements
CB = C // 128              # channel blocks (2)
HT = 7                     # output rows per tile
N_HT = H // HT             # 8 tiles per image
FREE = HT * WP             # 448: 7 padded rows read contiguously
SCALE, K = 1.0, 2
EPS = 1e-5
ALPHA = SCALE / K
EPS_EFF = EPS / (ALPHA * ALPHA)
M_TOT = float(N * H * W)   # BN reduction count (global batch)
NL_IMG_STRIDE = IMG        # per-image stride inside a channel block

F32 = mybir.dt.float32
F16 = mybir.dt.float16
BF16 = mybir.dt.bfloat16
FP8 = mybir.dt.float8e4
NP_FP8 = ml_dtypes.float8_e4m3
AF = mybir.ActivationFunctionType
ALU = mybir.AluOpType
DR = mybir.MatmulPerfMode.DoubleRow

_CACHE = {}


def _conv_phase(nc, pools, xbf, wt, y16, recs, copy_eng="scalar"):
    """One binarized conv3x3 over all local images + stats accumulation.

    xbf: [128, 2, NL*IMG] fp8 flat view of padded/binarized inputs
    wt:  [128, 3, 3, 2, C] fp8 weights ([ci, dy, dx, ci_blk, co] layout)
    y16: [cb] list of [128, NL, H, W] f16 outputs (exact integer sums)
    recs: [cb] list of [128, NL * N_HT, 6] f32 bn_stats records (one per
          tile; all tiles have equal counts, as bn_aggr's equal-weight merge
          requires)
    """
    psum = pools["psum"]
    for n in range(NL):
        for ht in range(N_HT):
            h0 = ht * HT
            idx = n * N_HT + ht
            for cob in range(CB):
                # out[h0+r, w] accumulates at pt column r*WP + w + 2 for all
                # nine (dy, dx): the dx shift is applied to the PSUM window so
                # every rhs read stays 16B-aligned (DoubleRow requirement).
                pt = psum.tile([128, FREE + 2], F32, tag="pt", name="pt")
                k = 0
                for dy in range(3):
                    for dx in range(3):
                        off = n * NL_IMG_STRIDE + (h0 + dy) * WP
                        nc.tensor.matmul(
                            pt[:, 2 - dx:FREE + 2 - dx],
                            wt[:, dy, dx, :, cob * 128:(cob + 1) * 128],
                            xbf[:, :, off:off + FREE],
                            start=(k == 0),
                            stop=(k == 8),
                            perf_mode=DR,
                        )
                        k += 1
                ptv = pt[:, 2:FREE + 2].rearrange("p (a b) -> p a b", b=WP)
                ysl = y16[cob][:, n, h0:h0 + HT, :]
                # fp32 PSUM -> exact integers in fp16
                if copy_eng == "scalar":
                    nc.scalar.activation(out=ysl, in_=ptv[:, :, 0:W], func=AF.Copy)
                else:
                    nc.vector.tensor_copy(ysl, ptv[:, :, 0:W])
                # one Welford record per tile (VectorE, contiguous fp16 read)
                yfl = y16[cob][:, n, :, :].rearrange("p a b -> p (a b)")
                nc.vector.bn_stats(
                    out=recs[cob][:, idx, :],
                    in_=yfl[:, h0 * W:(h0 + HT) * W])


def _stats_to_sums(nc, pools, recs, st):
    """bn_aggr per channel block, convert (mean, var) -> (sum, sumsq) local."""
    small = pools["small"]
    m_loc = float(NL * H * W)
    for cob in range(CB):
        mv = small.tile([128, 2], F32, tag=f"mv{cob}", name=f"mv{cob}")
        nc.vector.bn_aggr(out=mv[:], in_=recs[cob][:])
        msq = small.tile([128, 1], F32, tag=f"smsq{cob}", name=f"smsq{cob}")
        nc.vector.tensor_scalar_mul(st[:, 2 * cob:2 * cob + 1], mv[:, 0:1], m_loc)
        nc.vector.tensor_mul(msq[:], mv[:, 0:1], mv[:, 0:1])
        nc.vector.tensor_add(msq[:], msq[:], mv[:, 1:2])
        nc.vector.tensor_scalar_mul(st[:, 2 * cob + 1:2 * cob + 2], msq[:], m_loc)


def _bn_affine(nc, pools, gstats, gb, g_col, b_col, a_out, b_out):
    """Per-channel-block A/B:  A = g * rsqrt(var + eps_eff),  B = b - mean * A.

    gstats: [128, 4] globally-reduced (sum, sumsq) per channel block
    """
    small = pools["small"]
    epst = pools["epst"]
    for cob in range(CB):
        mean = small.tile([128, 1], F32, tag=f"mean{cob}", name=f"mean{cob}")
        ex2 = small.tile([128, 1], F32, tag=f"ex2{cob}", name=f"ex2{cob}")
        msq = small.tile([128, 1], F32, tag=f"msq{cob}", name=f"msq{cob}")
        var = small.tile([128, 1], F32, tag=f"var{cob}", name=f"var{cob}")
        rstd = small.tile([128, 1], F32, tag=f"rstd{cob}", name=f"rstd{cob}")
        nc.vector.tensor_scalar_mul(mean[:], gstats[:, 2 * cob:2 * cob + 1], 1.0 / M_TOT)
        nc.vector.tensor_scalar_mul(ex2[:], gstats[:, 2 * cob + 1:2 * cob + 2], 1.0 / M_TOT)
        # var = ex2 - mean^2
        nc.vector.tensor_mul(msq[:], mean[:], mean[:])
        nc.vector.tensor_sub(var[:], ex2[:], msq[:])
        # rstd = 1 / sqrt(var + eps_eff)
        nc.scalar.activation(out=rstd[:], in_=var[:], func=AF.Sqrt, bias=epst[:])
        nc.vector.reciprocal(rstd[:], rstd[:])
        # A = g * rstd ; B = b - mean * A
        nc.vector.tensor_mul(a_out[cob][:], gb[:, g_col + cob:g_col + cob + 1], rstd[:])
        nc.vector.tensor_mul(mean[:], mean[:], a_out[cob][:])
        nc.vector.tensor_sub(b_out[cob][:], gb[:, b_col + cob:b_col + cob + 1], mean[:])


def build():
    """Build + compile the per-core Bass program (SPMD, 8 cores)."""
    nc = bacc.Bacc("TRN2", target_bir_lowering=False, debug=False,
                   num_devices=N_CORES)

    x_in = nc.dram_tensor("x", [NL, C, H, W], F32, kind="ExternalInput").ap()
    w1_in = nc.dram_tensor("w1t", [3, 3, 128, 2, C], FP8, kind="ExternalInput").ap()
    w2_in = nc.dram_tensor("w2t", [3, 3, 128, 2, C], FP8, kind="ExternalInput").ap()
    gb_in = nc.dram_tensor("gb", [128, 8], F32, kind="ExternalInput").ap()
    out_d = nc.dram_tensor("out", [NL, C, H, W], F32, kind="ExternalOutput").ap()

    rg = [list(range(N_CORES))]

    with tile.TileContext(nc) as tc:
        import contextlib
        with contextlib.ExitStack() as ctx:
            consts = ctx.enter_context(tc.tile_pool(name="consts", bufs=1))
            xbp = ctx.enter_context(tc.tile_pool(name="xbp", bufs=1))
            y16p = ctx.enter_context(tc.tile_pool(name="y16p", bufs=1))
            statp = ctx.enter_context(tc.tile_pool(name="statp", bufs=1))
            small = ctx.enter_context(tc.tile_pool(name="small", bufs=1))
            psum = ctx.enter_context(tc.tile_pool(name="psum", bufs=6, space="PSUM"))
            dram = ctx.enter_context(tc.tile_pool(name="dram", bufs=1, space="DRAM"))
            epst = small.tile([128, 1], F32, tag="epst", name="epst")
            nc.vector.memset(epst[:], EPS_EFF)
            pools = {"psum": psum, "small": small, "epst": epst}

            # ---- dummy AllReduce: absorb first-collective setup cost under conv1
            dzero = small.tile([128, 1], F32, tag="dzero", name="dzero")
            nc.vector.memset(dzero[:], 0.0)
            d_in0 = dram.tile([128, 1], F32, tag="d_in0", name="d_in0")
            d_out0 = dram.tile([128 * N_CORES, 1], F32, tag="d_out0", name="d_out0")
            nc.sync.dma_start(out=d_in0[:], in_=dzero[:])
            nc.gpsimd.collective_compute(
                "AllGather", ALU.bypass, replica_groups=rg,
                ins=[d_in0.opt()], outs=[d_out0.opt()],
            )

            # ---- constants (w1 in its own pool, released after conv1)
            w1p = tc.tile_pool(name="w1p", bufs=1)
            w1pp = w1p.__enter__()
            w1t = w1pp.tile([128, 3, 3, 2, C], FP8, tag="w1t", name="w1t")
            nc.gpsimd.dma_start(
                out=w1t[:],
                in_=w1_in[:].rearrange("dy dx ci two co -> ci dy dx two co"))
            w2t = consts.tile([128, 3, 3, 2, C], FP8, tag="w2t", name="w2t")
            nc.gpsimd.dma_start(
                out=w2t[:],
                in_=w2_in[:].rearrange("dy dx ci two co -> ci dy dx two co"))
            gb = consts.tile([128, 8], F32, tag="gb", name="gb")
            nc.gpsimd.dma_start(out=gb[:], in_=gb_in[:])

            # ---- padded binarized activations (reused: conv1 input, then conv2
            # input).  Rows padded to a 64B pitch so DoubleRow rhs offsets stay
            # 16B-aligned; block stride NL*IMG = 14848 is 16B-aligned too.
            blk = NL * IMG
            assert blk % 16 == 0
            xb = xbp.tile([128, CB, blk], FP8, tag="xb", name="xb")
            xbf = xb[:]
            xbi = [xb[:, cib, :].rearrange(
                "p (n a b) -> p n a b", a=HP, b=WP) for cib in range(CB)]
            # zero only the halo borders + pitch padding (interior is always
            # overwritten by the sign activations before it is read)
            for cib in range(CB):
                nc.vector.memset(xbi[cib][:, :, 0, :], 0.0)
                nc.vector.memset(xbi[cib][:, :, H + 1, :], 0.0)
                nc.vector.memset(xbi[cib][:, :, 1:H + 1, 0:1], 0.0)
                nc.vector.memset(xbi[cib][:, :, 1:H + 1, W + 1:WP], 0.0)

            # ---- conv outputs as exact integers (reused for conv1 then conv2)
            y16 = [y16p.tile([128, NL, H, W], F16, tag=f"y16_{cob}", name=f"y16_{cob}")
                   for cob in range(CB)]

            # ---- bn_stats records
            r1c = [statp.tile([128, NL * N_HT, 6], F32, tag=f"r1c{c}", name=f"r1c{c}") for c in range(CB)]
            r2c = [statp.tile([128, NL * N_HT, 6], F32, tag=f"r2c{c}", name=f"r2c{c}") for c in range(CB)]

            # ---- phase 0: load x, binarize into padded buffers
            with tc.tile_pool(name="stage", bufs=3) as stage:
                dma_rr = [nc.sync]
                for n in range(NL):
                    for cib in range(CB):
                        xs = stage.tile([128, H, W], F32, tag="xstage", name="xstage")
                        dma_rr[0].dma_start(
                            out=xs[:], in_=x_in[n, cib * 128:(cib + 1) * 128, :, :])
                        nc.scalar.activation(
                            out=xbi[cib][:, n, 1:H + 1, 1:W + 1], in_=xs[:],
                            func=AF.Sign)

            # ---- conv1 + stats
            _conv_phase(nc, pools, xbf, w1t, y16, r1c)
            w1p.__exit__(None, None, None)

            # ---- residual prefetch pool: reuses the stage/w1 zones; DMAs run
            # during phase2/conv2 while the DMA engines are otherwise idle
            resp = ctx.enter_context(tc.tile_pool(name="resp", bufs=6))
            youtp = ctx.enter_context(tc.tile_pool(name="youtp", bufs=2))

            # ---- aggregate + AllReduce stats 1
            st1 = small.tile([128, 4], F32, tag="st1", name="st1")
            _stats_to_sums(nc, pools, r1c, st1)
            d_in1 = dram.tile([128, 4], F32, tag="d_in1", name="d_in1")
            d_out1 = dram.tile([128 * N_CORES, 4], F32, tag="d_out1", name="d_out1")
            nc.gpsimd.dma_start(out=d_in1[:], in_=st1[:])
            nc.gpsimd.collective_compute(
                "AllGather", ALU.bypass, replica_groups=rg,
                ins=[d_in1.opt()], outs=[d_out1.opt()],
            )
            gag1 = small.tile([128, 4, N_CORES], F32, tag="gag1", name="gag1")
            nc.gpsimd.dma_start(
                out=gag1[:], in_=d_out1[:].rearrange("(r p) c -> p c r", p=128))
            gstats1 = small.tile([128, 4], F32, tag="gstats1", name="gstats1")
            nc.vector.reduce_sum(gstats1[:], gag1[:], axis=mybir.AxisListType.X)

            a1 = [small.tile([128, 1], F32, tag=f"a1_{c}", name=f"a1_{c}") for c in range(CB)]
            b1 = [small.tile([128, 1], F32, tag=f"b1_{c}", name=f"b1_{c}") for c in range(CB)]
            _bn_affine(nc, pools, gstats1, gb, g_col=0, b_col=2, a_out=a1, b_out=b1)

            # ---- phase 2: out1 = sign(A1 * y1 + B1) into the padded buffers
            for n in range(NL):
                for ht in range(N_HT):
                    h0 = ht * HT
                    for cob in range(CB):
                        nc.scalar.activation(
                            out=xbi[cob][:, n, h0 + 1:h0 + HT + 1, 1:W + 1],
                            in_=y16[cob][:, n, h0:h0 + HT, :],
                            func=AF.Sign,
                            scale=a1[cob][:],
                            bias=b1[cob][:],
                        )

            # ---- residual prefetch (DMA only; overlaps phase2 + conv2)
            xres = []
            dma_rr2 = [nc.sync]
            for n in range(NL):
                for cib in range(CB):
                    xr = resp.tile([128, H, W], F32, tag="xres", name="xres")
                    dma_rr2[0].dma_start(
                        out=xr[:], in_=x_in[n, cib * 128:(cib + 1) * 128, :, :])
                    xres.append(xr)

            # ---- conv2 + stats (y16 overwritten with conv2 integer sums).
            # ScalarE is busy with the phase-2 signs here, so the PSUM copy
            # rides on VectorE instead.
            _conv_phase(nc, pools, xbf, w2t, y16, r2c, copy_eng="vector")

            # ---- aggregate + AllReduce stats 2
            st2 = small.tile([128, 4], F32, tag="st2", name="st2")
            _stats_to_sums(nc, pools, r2c, st2)
            d_in2 = dram.tile([128, 4], F32, tag="d_in2", name="d_in2")
            d_out2 = dram.tile([128 * N_CORES, 4], F32, tag="d_out2", name="d_out2")
            nc.gpsimd.dma_start(out=d_in2[:], in_=st2[:])
            nc.gpsimd.collective_compute(
                "AllGather", ALU.bypass, replica_groups=rg,
                ins=[d_in2.opt()], outs=[d_out2.opt()],
            )
            gag2 = small.tile([128, 4, N_CORES], F32, tag="gag2", name="gag2")
            nc.gpsimd.dma_start(
                out=gag2[:], in_=d_out2[:].rearrange("(r p) c -> p c r", p=128))
            gstats2 = small.tile([128, 4], F32, tag="gstats2", name="gstats2")
            nc.vector.reduce_sum(gstats2[:], gag2[:], axis=mybir.AxisListType.X)

            a2 = [small.tile([128, 1], F32, tag=f"a2_{c}", name=f"a2_{c}") for c in range(CB)]
            b2 = [small.tile([128, 1], F32, tag=f"b2_{c}", name=f"b2_{c}") for c in range(CB)]
            _bn_affine(nc, pools, gstats2, gb, g_col=4, b_col=6, a_out=a2, b_out=b2)

            # ---- final: out = clip(A2 * y2 + B2 + x, -1, 1)
            for n in range(NL):
                for cib in range(CB):
                    xr = xres[n * CB + cib]
                    yout = youtp.tile([128, H, W], F32, tag="yout", name="yout")
                    nc.scalar.activation(
                        out=yout[:], in_=y16[cib][:, n, :, :], func=AF.Identity,
                        scale=a2[cib][:], bias=b2[cib][:])
                    nc.vector.tensor_add(yout[:], yout[:], xr[:])
                    nc.vector.tensor_scalar(
                        out=yout[:], in0=yout[:], scalar1=1.0, scalar2=-1.0,
                        op0=ALU.min, op1=ALU.max)
                    nc.sync.dma_start(
                        out=out_d[n, cib * 128:(cib + 1) * 128, :, :], in_=yout[:])

    nc.compile()
    return nc


def _prep_inputs(x, w1, g1, b1, w2, g2, b2):
    """Host-side sharding + weight layout. Returns per-core input maps."""
    x = np.ascontiguousarray(np.asarray(x, dtype=np.float32))

    # sign(w) in [dy, dx, ci%128, ci//128, co] fp8 layout; +-1/0 exact
    def prep_w(w):
        wt = np.sign(np.asarray(w, np.float32)).transpose(2, 3, 1, 0)  # dy dx ci co
        wt = wt.reshape(3, 3, 2, 128, C).transpose(0, 1, 3, 2, 4)      # dy dx 128 2 co
        return np.ascontiguousarray(wt).astype(NP_FP8)

    w1t = prep_w(w1)
    w2t = prep_w(w2)
    gb = np.stack(
        [np.asarray(v, np.float32)[c * 128:(c + 1) * 128]
         for v in (g1, b1, g2, b2) for c in range(CB)],
        axis=1,
    )
    # column order: g1_0 g1_1 b1_0 b1_1 g2_0 g2_1 b2_0 b2_1
    gb = np.ascontiguousarray(gb)
    in_maps = []
    for c in range(N_CORES):
        in_maps.append({
            "x": x[c * NL:(c + 1) * NL],
            "w1t": w1t,
            "w2t": w2t,
            "gb": gb,
        })
    return in_maps


def run(inputs, trace=False):
    """Run the kernel on 8 cores; returns (full_output, BassKernelResults)."""
    if "nc" not in _CACHE:
        _CACHE["nc"] = build()
    nc = _CACHE["nc"]
    in_maps = _prep_inputs(**inputs)
    res = bass_utils.run_bass_kernel_spmd(
        nc, in_maps, core_ids=list(range(N_CORES)), trace=trace)
    out = np.concatenate([res.results[c]["out"] for c in range(N_CORES)], axis=0)
    return out, res


def kernel(**inputs):
    out, _ = run(inputs, trace=False)
    return out



# revision 7
# speedup vs baseline: 1.2410x; 1.2410x over previous
"""Trainium2 Bass kernel for a binarized (1w1a) BasicBlock:

    out1 = hardtanh(BN1(binconv(x, w1)))          # BN in training mode (batch stats)
    out  = hardtanh(BN2(binconv(out1, w2)) + x)   # identity shortcut

binconv(x, w) = conv3x3(sign(x), sign(w), pad=1) * (SCALE / K)

Sharding: data-parallel over batch (4 images per core on 8 cores), weights
replicated.  BN batch statistics (per-channel sum and sum-of-squares) are
combined with tiny cross-core AllReduces.

Implementation notes:
  - sign() values (+-1, 0) are exact in fp8, and the 3x3x256 conv
    accumulates integers |S| <= 2304 in fp32 PSUM, so the convolutions are
    bit-exact at fp8 TensorE rate.  The SCALE/K factor commutes through
    BatchNorm and is folded into eps:  eps_eff = eps / (SCALE/K)^2.
  - conv3x3 = 9 shifted matmuls accumulated in PSUM, with fp8 DoubleRow
    contracting both 128-channel blocks per pass.  Activations live in SBUF
    as [128 ch-pair, 2, n, 58, 64] zero-padded images, so every shift is an
    access-pattern offset and every rhs read stays 16B-aligned.
  - Weights are pre-transposed on the host to the exact SBUF layout
    ([ci%128, dy, dx, ci//128, co]) so the weight DMA is contiguous.
  - x is loaded from HBM once; the staged fp32 tile feeds both the sign
    activation (ScalarE -> fp8 conv input) and an fp16 residual copy
    (Pool engine) kept resident for the final shortcut add.
  - BN statistics are reduced in two chunks per conv: images {0,1,2} are
    AllReduced while the conv is still working on image 3, so only the
    image-3 chunk's collective latency is exposed at the phase boundary.
  - The final phase runs in fp16 (exact conv integers + ~1e-3 rounding,
    well inside tolerance): ScalarE applies the BN affine, DVE adds the
    residual, DVE/Pool split the hardtanh clip, and the output is DMA'd
    at fp16 (host converts to fp32).
"""

import numpy as np
import ml_dtypes

import concourse.bass as bass
import concourse.tile as tile
from concourse import bacc, mybir
from concourse import bass_utils

N_CORES = 8
N, C, H, W = 32, 256, 56, 56
NL = N // N_CORES          # images per core
HP = H + 2                 # padded rows (58)
WP = 64                    # padded row pitch: keeps DoubleRow offsets 16B-aligned
IMG = HP * WP              # padded image elements
CB = C // 128              # channel blocks (2)
HT = 7                     # output rows per tile
N_HT = H // HT             # 8 tiles per image
FREE = HT * WP             # 448: 7 padded rows read contiguously
SCALE, K = 1.0, 2
EPS = 1e-5
ALPHA = SCALE / K
EPS_EFF = EPS / (ALPHA * ALPHA)
M_TOT = float(N * H * W)   # BN reduction count (global batch)
HW_ = H * W

F32 = mybir.dt.float32
F16 = mybir.dt.float16
FP8 = mybir.dt.float8e4
NP_FP8 = ml_dtypes.float8_e4m3
AF = mybir.ActivationFunctionType
ALU = mybir.AluOpType
DR = mybir.MatmulPerfMode.DoubleRow

GROUP_A = [0, 1, 2]        # stats chunk reduced while conv still runs
GROUP_B = [3]              # stats chunk on the critical path

_CACHE = {}


def _conv_img(nc, psum, xbf, wt, y16, recs, n):
    """One binarized conv3x3 over local image n + its stats record."""
    for ht in range(N_HT):
        h0 = ht * HT
        for cob in range(CB):
            # out[h0+r, w] accumulates at pt column r*WP + w + 2 for all
            # nine (dy, dx): the dx shift is applied to the PSUM window so
            # every rhs read stays 16B-aligned (DoubleRow requirement).
            pt = psum.tile([128, FREE + 2], F32, tag="pt", name="pt")
            k = 0
            for dy in range(3):
                for dx in range(3):
                    off = n * IMG + (h0 + dy) * WP
                    nc.tensor.matmul(
                        pt[:, 2 - dx:FREE + 2 - dx],
                        wt[:, dy, dx, :, cob * 128:(cob + 1) * 128],
                        xbf[:, :, off:off + FREE],
                        start=(k == 0),
                        stop=(k == 8),
                        perf_mode=DR,
                    )
                    k += 1
            ptv = pt[:, 2:FREE + 2].rearrange("p (a b) -> p a b", b=WP)
            # fp32 PSUM -> exact integers in fp16
            nc.scalar.activation(
                out=y16[cob][:, n, h0:h0 + HT, :], in_=ptv[:, :, 0:W],
                func=AF.Copy)
            # one Welford record per tile (VectorE, contiguous fp16 read;
            # the 512-element bn_stats HW limit forbids whole-image records)
            yfl = y16[cob][:, n, :, :].rearrange("p a b -> p (a b)")
            nc.vector.bn_stats(
                out=recs[cob][:, n, ht, :],
                in_=yfl[:, h0 * W:(h0 + HT) * W])


def _group_allreduce(nc, pools, recs, images, tagp):
    """Aggregate bn_stats for a group of images into (sum, sumsq) per
    channel block and AllReduce across the 8 cores.  Returns the SBUF tile
    that will hold the globally-summed [128, 4] stats."""
    small, dram = pools["small"], pools["dram"]
    rg = [list(range(N_CORES))]
    m_loc = float(len(images) * HW_)
    st = small.tile([128, 4], F32, tag=f"st{tagp}", name=f"st{tagp}")
    lo, hi = images[0], images[-1] + 1
    for cob in range(CB):
        mv = small.tile([128, 2], F32, tag=f"mv{tagp}{cob}", name=f"mv{tagp}{cob}")
        rv = recs[cob][:, lo:hi, :, :].rearrange("p n t s -> p (n t) s")
        nc.vector.bn_aggr(out=mv[:], in_=rv)
        msq = small.tile([128, 1], F32, tag=f"sq{tagp}{cob}", name=f"sq{tagp}{cob}")
        nc.vector.tensor_scalar_mul(st[:, 2 * cob:2 * cob + 1], mv[:, 0:1], m_loc)
        nc.vector.tensor_mul(msq[:], mv[:, 0:1], mv[:, 0:1])
        nc.vector.tensor_add(msq[:], msq[:], mv[:, 1:2])
        nc.vector.tensor_scalar_mul(st[:, 2 * cob + 1:2 * cob + 2], msq[:], m_loc)
    d_in = dram.tile([128, 4], F32, tag=f"din{tagp}", name=f"din{tagp}")
    d_out = dram.tile([128, 4], F32, tag=f"dout{tagp}", name=f"dout{tagp}")
    nc.sync.dma_start(out=d_in[:], in_=st[:])
    nc.gpsimd.collective_compute(
        "AllReduce", ALU.add, replica_groups=rg,
        ins=[d_in.opt()], outs=[d_out.opt()],
    )
    gsum = small.tile([128, 4], F32, tag=f"gs{tagp}", name=f"gs{tagp}")
    nc.sync.dma_start(out=gsum[:], in_=d_out[:])
    return gsum


def _bn_affine(nc, pools, gstats, gb, g_col, b_col, a_out, b_out):
    """Per-channel-block A/B:  A = g * rsqrt(var + eps_eff),  B = b - mean * A.

    gstats: [128, 4] globally-reduced (sum, sumsq) per channel block
    """
    small = pools["small"]
    epst = pools["epst"]
    for cob in range(CB):
        mean = small.tile([128, 1], F32, tag=f"mean{cob}", name=f"mean{cob}")
        ex2 = small.tile([128, 1], F32, tag=f"ex2{cob}", name=f"ex2{cob}")
        msq = small.tile([128, 1], F32, tag=f"msq{cob}", name=f"msq{cob}")
        var = small.tile([128, 1], F32, tag=f"var{cob}", name=f"var{cob}")
        rstd = small.tile([128, 1], F32, tag=f"rstd{cob}", name=f"rstd{cob}")
        nc.vector.tensor_scalar_mul(mean[:], gstats[:, 2 * cob:2 * cob + 1], 1.0 / M_TOT)
        nc.vector.tensor_scalar_mul(ex2[:], gstats[:, 2 * cob + 1:2 * cob + 2], 1.0 / M_TOT)
        # var = ex2 - mean^2
        nc.vector.tensor_mul(msq[:], mean[:], mean[:])
        nc.vector.tensor_sub(var[:], ex2[:], msq[:])
        # rstd = 1 / sqrt(var + eps_eff)
        nc.scalar.activation(out=rstd[:], in_=var[:], func=AF.Sqrt, bias=epst[:])
        nc.vector.reciprocal(rstd[:], rstd[:])
        # A = g * rstd ; B = b - mean * A
        nc.vector.tensor_mul(a_out[cob][:], gb[:, g_col + cob:g_col + cob + 1], rstd[:])
        nc.vector.tensor_mul(mean[:], mean[:], a_out[cob][:])
        nc.vector.tensor_sub(b_out[cob][:], gb[:, b_col + cob:b_col + cob + 1], mean[:])


def build():
    """Build + compile the per-core Bass program (SPMD, 8 cores)."""
    nc = bacc.Bacc("TRN2", target_bir_lowering=False, debug=False,
                   num_devices=N_CORES)

    x_in = nc.dram_tensor("x", [NL, C, H, W], F32, kind="ExternalInput").ap()
    w1_in = nc.dram_tensor("w1t", [128, 3, 3, 2, C], FP8, kind="ExternalInput").ap()
    w2_in = nc.dram_tensor("w2t", [128, 3, 3, 2, C], FP8, kind="ExternalInput").ap()
    gb_in = nc.dram_tensor("gb", [128, 8], F32, kind="ExternalInput").ap()
    out_d = nc.dram_tensor("out", [NL, C, H, W], F16, kind="ExternalOutput").ap()

    rg = [list(range(N_CORES))]

    with tile.TileContext(nc) as tc:
        import contextlib
        with contextlib.ExitStack() as ctx:
            consts = ctx.enter_context(tc.tile_pool(name="consts", bufs=1))
            xbp = ctx.enter_context(tc.tile_pool(name="xbp", bufs=1))
            y16p = ctx.enter_context(tc.tile_pool(name="y16p", bufs=1))
            xresp = ctx.enter_context(tc.tile_pool(name="xresp", bufs=1))
            statp = ctx.enter_context(tc.tile_pool(name="statp", bufs=1))
            small = ctx.enter_context(tc.tile_pool(name="small", bufs=1))
            psum = ctx.enter_context(tc.tile_pool(name="psum", bufs=6, space="PSUM"))
            dram = ctx.enter_context(tc.tile_pool(name="dram", bufs=1, space="DRAM"))
            stage = ctx.enter_context(tc.tile_pool(name="stage", bufs=3))
            youtp = ctx.enter_context(tc.tile_pool(name="youtp", bufs=3))
            epst = small.tile([128, 1], F32, tag="epst", name="epst")
            nc.vector.memset(epst[:], EPS_EFF)
            pools = {"small": small, "epst": epst, "dram": dram}

            # ---- dummy AllReduce: absorb first-collective setup cost
            dzero = small.tile([128, 1], F32, tag="dzero", name="dzero")
            nc.vector.memset(dzero[:], 0.0)
            d_in0 = dram.tile([128, 1], F32, tag="d_in0", name="d_in0")
            d_out0 = dram.tile([128, 1], F32, tag="d_out0", name="d_out0")
            nc.sync.dma_start(out=d_in0[:], in_=dzero[:])
            nc.gpsimd.collective_compute(
                "AllReduce", ALU.add, replica_groups=rg,
                ins=[d_in0.opt()], outs=[d_out0.opt()],
            )

            # ---- constants (host already in SBUF layout: contiguous DMAs)
            w1t = consts.tile([128, 3, 3, 2, C], FP8, tag="w1t", name="w1t")
            nc.gpsimd.dma_start(out=w1t[:], in_=w1_in[:])
            w2t = consts.tile([128, 3, 3, 2, C], FP8, tag="w2t", name="w2t")
            nc.gpsimd.dma_start(out=w2t[:], in_=w2_in[:])
            gb = consts.tile([128, 8], F32, tag="gb", name="gb")
            nc.gpsimd.dma_start(out=gb[:], in_=gb_in[:])

            # ---- padded binarized activations (reused: conv1 input, then conv2
            # input).  Rows padded to a 64B pitch so DoubleRow rhs offsets stay
            # 16B-aligned.
            blk = NL * IMG
            xb = xbp.tile([128, CB, blk], FP8, tag="xb", name="xb")
            xbf = xb[:]
            xbi = [xb[:, cib, :].rearrange(
                "p (n a b) -> p n a b", a=HP, b=WP) for cib in range(CB)]
            # zero only the halo borders + pitch padding (interior is always
            # overwritten by the sign activations before it is read)
            for cib in range(CB):
                nc.vector.memset(xbi[cib][:, :, 0, :], 0.0)
                nc.vector.memset(xbi[cib][:, :, H + 1, :], 0.0)
                nc.vector.memset(xbi[cib][:, :, 1:H + 1, 0:1], 0.0)
                nc.vector.memset(xbi[cib][:, :, 1:H + 1, W + 1:WP], 0.0)

            # ---- conv outputs as exact integers (reused for conv1 then conv2)
            y16 = [y16p.tile([128, NL, H, W], F16, tag=f"y16_{cob}", name=f"y16_{cob}")
                   for cob in range(CB)]
            # ---- fp16 residual copy of x, kept resident for the final add
            xres = xresp.tile([128, CB, NL, H, W], F16, tag="xres", name="xres")

            # ---- bn_stats records (one per row-tile, batched per image)
            r1c = [statp.tile([128, NL, N_HT, 6], F32, tag=f"r1c{c}", name=f"r1c{c}")
                   for c in range(CB)]
            r2c = [statp.tile([128, NL, N_HT, 6], F32, tag=f"r2c{c}", name=f"r2c{c}")
                   for c in range(CB)]

            # ---- phase 0: load x, binarize + fp16 copy
            def phase0(n):
                for cib in range(CB):
                    xs = stage.tile([128, H, W], F32, tag="xstage", name="xstage")
                    nc.sync.dma_start(
                        out=xs[:], in_=x_in[n, cib * 128:(cib + 1) * 128, :, :])
                    nc.scalar.activation(
                        out=xbi[cib][:, n, 1:H + 1, 1:W + 1], in_=xs[:],
                        func=AF.Sign)
                    nc.gpsimd.tensor_copy(xres[:, cib, n, :, :], xs[:])

            # ---- conv1 (phase0 interleaved so DMA/signs lead the matmuls)
            phase0(0)
            phase0(1)
            _conv_img(nc, psum, xbf, w1t, y16, r1c, 0)
            phase0(2)
            _conv_img(nc, psum, xbf, w1t, y16, r1c, 1)
            phase0(3)
            _conv_img(nc, psum, xbf, w1t, y16, r1c, 2)
            gsum1a = _group_allreduce(nc, pools, r1c, GROUP_A, "1a")
            _conv_img(nc, psum, xbf, w1t, y16, r1c, 3)
            gsum1b = _group_allreduce(nc, pools, r1c, GROUP_B, "1b")

            gstats1 = small.tile([128, 4], F32, tag="gstats1", name="gstats1")
            nc.vector.tensor_add(gstats1[:], gsum1a[:], gsum1b[:])
            a1 = [small.tile([128, 1], F32, tag=f"a1_{c}", name=f"a1_{c}") for c in range(CB)]
            b1 = [small.tile([128, 1], F32, tag=f"b1_{c}", name=f"b1_{c}") for c in range(CB)]
            _bn_affine(nc, pools, gstats1, gb, g_col=0, b_col=2, a_out=a1, b_out=b1)

            # ---- phase 2 + conv2, pipelined per image:
            # out1 = sign(A1 * y1 + B1) written back into the padded buffers
            def sign2(n):
                for cob in range(CB):
                    nc.scalar.activation(
                        out=xbi[cob][:, n, 1:H + 1, 1:W + 1],
                        in_=y16[cob][:, n, :, :],
                        func=AF.Sign,
                        scale=a1[cob][:],
                        bias=b1[cob][:],
                    )

            sign2(0)
            sign2(1)
            _conv_img(nc, psum, xbf, w2t, y16, r2c, 0)
            sign2(2)
            _conv_img(nc, psum, xbf, w2t, y16, r2c, 1)
            sign2(3)
            _conv_img(nc, psum, xbf, w2t, y16, r2c, 2)
            gsum2a = _group_allreduce(nc, pools, r2c, GROUP_A, "2a")
            _conv_img(nc, psum, xbf, w2t, y16, r2c, 3)
            gsum2b = _group_allreduce(nc, pools, r2c, GROUP_B, "2b")

            gstats2 = small.tile([128, 4], F32, tag="gstats2", name="gstats2")
            nc.vector.tensor_add(gstats2[:], gsum2a[:], gsum2b[:])
            a2 = [small.tile([128, 1], F32, tag=f"a2_{c}", name=f"a2_{c}") for c in range(CB)]
            b2 = [small.tile([128, 1], F32, tag=f"b2_{c}", name=f"b2_{c}") for c in range(CB)]
            _bn_affine(nc, pools, gstats2, gb, g_col=4, b_col=6, a_out=a2, b_out=b2)

            # ---- final: out = clip(A2 * y2 + B2 + x, -1, 1), all fp16.
            # ScalarE does the affine, DVE the residual add; the clip is
            # split between DVE and the Pool engine to balance the tail.
            for i, (n, cib) in enumerate([(n, c) for n in range(NL) for c in range(CB)]):
                yout = youtp.tile([128, H, W], F16, tag="yout", name="yout")
                nc.scalar.activation(
                    out=yout[:], in_=y16[cib][:, n, :, :], func=AF.Identity,
                    scale=a2[cib][:], bias=b2[cib][:])
                nc.vector.tensor_add(yout[:], yout[:], xres[:, cib, n, :, :])
                clip_eng = nc.vector if i % 8 < 5 else nc.gpsimd
                clip_eng.tensor_scalar(
                    out=yout[:], in0=yout[:], scalar1=1.0, scalar2=-1.0,
                    op0=ALU.min, op1=ALU.max)
                nc.sync.dma_start(
                    out=out_d[n, cib * 128:(cib + 1) * 128, :, :], in_=yout[:])

    nc.compile()
    return nc


def _prep_inputs(x, w1, g1, b1, w2, g2, b2):
    """Host-side sharding + weight layout. Returns per-core input maps."""
    x = np.ascontiguousarray(np.asarray(x, dtype=np.float32))

    # sign(w) pre-transposed to the SBUF layout [ci%128, dy, dx, ci//128, co]
    # (ci = k*128 + p), so the device DMA is fully contiguous; +-1/0 exact
    def prep_w(w):
        wt = np.sign(np.asarray(w, np.float32)).transpose(1, 2, 3, 0)  # ci dy dx co
        wt = wt.reshape(2, 128, 3, 3, C).transpose(1, 2, 3, 0, 4)      # p dy dx k co
        return np.ascontiguousarray(wt).astype(NP_FP8)

    w1t = prep_w(w1)
    w2t = prep_w(w2)
    gb = np.stack(
        [np.asarray(v, np.float32)[c * 128:(c + 1) * 128]
         for v in (g1, b1, g2, b2) for c in range(CB)],
        axis=1,
    )
    # column order: g1_0 g1_1 b1_0 b1_1 g2_0 g2_1 b2_0 b2_1
    gb = np.ascontiguousarray(gb)
    in_maps = []
    for c in range(N_CORES):
        in_maps.append({
            "x": x[c * NL:(c + 1) * NL],
            "w1t": w1t,
            "w2t": w2t,
            "gb": gb,
        })
    return in_maps


def run(inputs, trace=False):
    """Run the kernel on 8 cores; returns (full_output, BassKernelResults)."""
    if "nc" not in _CACHE:
        _CACHE["nc"] = build()
    nc = _CACHE["nc"]
    in_maps = _prep_inputs(**inputs)
    res = bass_utils.run_bass_kernel_spmd(
        nc, in_maps, core_ids=list(range(N_CORES)), trace=trace)
    out = np.concatenate(
        [res.results[c]["out"].astype(np.float32) for c in range(N_CORES)], axis=0)
    return out, res


def kernel(**inputs):
    out, _ = run(inputs, trace=False)
    return out
